# revision 1
# baseline (speedup 1.0000x reference)
"""FP8Linear (dynamic per-tensor fp8 quantized linear) on 8 Trainium2 cores.

Strategy (tensor-parallel, quantization sharded across cores):
  - Core c quantizes x rows [c*2048:(c+1)*2048] and w rows [c*1024:(c+1)*1024].
  - Launch A: per-core partial abs-max of its x / w blocks; host reduces the
    partials and computes quantization scales bit-exactly in f32 (TRN has no
    IEEE f32 divide instruction, and the scale must match the jnp reference
    exactly to keep the same e4m3 rounding grid).
  - Launch B: core c PE-transposes its x block (f32), quantizing on PSUM
    evacuation to fp8 at HALF the reference scale (TRN e4m3 saturates at 240
    vs OCP e4m3fn's 448; half scale keeps the same rounding grid, fixed by a
    4x factor folded into the output scale).  Its w slice goes the same way
    into DRAM and is AllGathered (2 MB/rank, issued first so it hides behind
    the x work).  Then a DoubleRow fp8 matmul computes the full output-feature
    dim for the core's own x rows, with a fused (psum*s + bias) -> fp16
    epilogue.  x^T stays SBUF-resident; w^T streams from the gather.
  - Host stacks the 8 row-blocks.
"""
import os
import sys

for _p in ("/opt/trn_rl_repo", "/root/.axon_site/_ro/trn_rl_repo"):
    if _p not in sys.path and os.path.isdir(_p):
        sys.path.append(_p)

import numpy as np

import concourse.bass as bass  # noqa: F401
from concourse import bacc, bass_isa
import concourse.mybir as mybir
import concourse.tile as tile
from concourse.bass_utils import run_bass_kernel_spmd
from concourse.masks import make_identity

F32 = mybir.dt.float32
F16 = mybir.dt.float16
FP8 = mybir.dt.float8e4

N_CORES = 8
M_FULL, K, N_FULL = 16384, 2048, 8192
M_LOC = M_FULL // N_CORES            # 2048 x-rows per core
N_LOC = N_FULL // N_CORES            # 1024 w-rows quantized per core
KSUB = K // 128                      # 16
N_TILE = 512                         # psum free dim
N_HALF = N_LOC // 2                  # AllGather split granularity
M_SPLIT = 2                          # x halves for phase2/phase3 overlap
M_HALF = M_LOC // M_SPLIT            # 1024

TRACE = False
LAST_EXEC_NS = []


def _build_amax():
    """Launch A: per-core partial absmax -> [1, 2] f32 (x part, w part)."""
    nc = bacc.Bacc("TRN2", target_bir_lowering=False, debug=False,
                   num_devices=N_CORES)
    xs = nc.dram_tensor("xs", [M_LOC, K], F32, kind="ExternalInput")
    wl = nc.dram_tensor("wl", [N_LOC, K], F32, kind="ExternalInput")
    amax_out = nc.dram_tensor("amax_out", [1, 2], F32, kind="ExternalOutput")

    n_xs = M_LOC // 128
    n_ws = N_LOC // 128
    with tile.TileContext(nc) as tc:
        with (
            tc.tile_pool(name="stripes", bufs=4) as sp,
            tc.tile_pool(name="stats", bufs=1) as st,
        ):
            pm = st.tile([128, n_xs + n_ws], F32)
            for i in range(n_xs):
                s = sp.tile([128, K], F32, tag="s")
                nc.sync.dma_start(s[:], xs[i * 128:(i + 1) * 128, :])
                nc.vector.tensor_reduce(
                    pm[:, i:i + 1], s[:], axis=mybir.AxisListType.X,
                    op=mybir.AluOpType.max, apply_absolute_value=True)
            for i in range(n_ws):
                s = sp.tile([128, K], F32, tag="s")
                nc.sync.dma_start(s[:], wl[i * 128:(i + 1) * 128, :])
                nc.vector.tensor_reduce(
                    pm[:, n_xs + i:n_xs + i + 1], s[:], axis=mybir.AxisListType.X,
                    op=mybir.AluOpType.max, apply_absolute_value=True)
            red = st.tile([128, 2], F32)
            nc.vector.tensor_reduce(
                red[:, 0:1], pm[:, 0:n_xs], axis=mybir.AxisListType.X,
                op=mybir.AluOpType.max)
            nc.vector.tensor_reduce(
                red[:, 1:2], pm[:, n_xs:n_xs + n_ws], axis=mybir.AxisListType.X,
                op=mybir.AluOpType.max)
            allred = st.tile([128, 2], F32)
            nc.gpsimd.partition_all_reduce(
                allred[:], red[:], channels=128, reduce_op=bass_isa.ReduceOp.max)
            nc.sync.dma_start(amax_out[:], allred[0:1, :])
    nc.compile()
    return nc


def _build_main():
    """Launch B: quantize + transpose, w AllGather, DoubleRow matmul."""
    nc = bacc.Bacc("TRN2", target_bir_lowering=False, debug=False,
                   num_devices=N_CORES)
    xs = nc.dram_tensor("xs", [M_LOC, K], F32, kind="ExternalInput")
    wl = nc.dram_tensor("wl", [N_LOC, K], F32, kind="ExternalInput")
    bias_in = nc.dram_tensor("bias_in", [1, N_FULL], F16, kind="ExternalInput")
    scales = nc.dram_tensor("scales", [1, 4], F32, kind="ExternalInput")
    out = nc.dram_tensor("out", [M_LOC, N_FULL], F16, kind="ExternalOutput")

    # w^T fp8 halves (AllGather input must be internal-Local, output Shared)
    wT_loc = [nc.dram_tensor(f"wT_loc{h}", [K, N_HALF], FP8) for h in range(2)]
    wT_all = [nc.dram_tensor(f"wT_all{h}", [N_CORES, K, N_HALF], FP8,
                             addr_space="Shared") for h in range(2)]
    wT_loc_v = [t.ap().rearrange("(ko p) n -> p ko n", p=128) for t in wT_loc]

    with tile.TileContext(nc) as tc:
        with (
            tc.tile_pool(name="const", bufs=1) as cp,
            tc.tile_pool(name="stripe", bufs=6) as sp,
            tc.tile_pool(name="tp", bufs=2, space="PSUM") as tpp,
            tc.tile_pool(name="wasm", bufs=2) as wap,
            tc.tile_pool(name="xres", bufs=1) as xrp,
            tc.tile_pool(name="wt", bufs=4) as wtp,
            tc.tile_pool(name="mm", bufs=6, space="PSUM") as mp,
            tc.tile_pool(name="ep", bufs=4) as epp,
        ):
            ident = cp.tile([128, 128], F32)
            make_identity(nc, ident[:])
            sc_row = cp.tile([1, 4], F32)
            nc.sync.dma_start(sc_row[:], scales[:])
            sc = cp.tile([128, 4], F32)
            nc.gpsimd.partition_broadcast(sc[:], sc_row[:], channels=128)
            # --- w first: quantize+transpose into wT_loc halves, AllGather
            # each half ASAP so the gathers hide behind the x transposes ---
            for h in range(2):
                wa = wap.tile([128, KSUB, N_HALF], FP8, tag="wa")
                for ns in range(N_HALF // 128):
                    row0 = h * N_HALF + ns * 128
                    s = sp.tile([128, K], F32, tag="stripe")
                    nc.sync.dma_start(s[:], wl[row0:row0 + 128, :])
                    for kc in range(KSUB):
                        t = tpp.tile([128, 128], F32, tag="t")
                        nc.tensor.transpose(
                            t[:], s[:, kc * 128:(kc + 1) * 128], ident[:])
                        nc.vector.tensor_scalar_mul(
                            wa[:, kc, ns * 128:(ns + 1) * 128], t[:], sc[:, 1:2])
                nc.sync.dma_start(wT_loc_v[h][:], wa[:])
                nc.gpsimd.collective_compute(
                    "AllGather", mybir.AluOpType.bypass,
                    replica_groups=[list(range(N_CORES))],
                    ins=[wT_loc[h].ap().opt()], outs=[wT_all[h].ap().opt()])

            bias_row = cp.tile([1, N_FULL], F16)
            nc.sync.dma_start(bias_row[:], bias_in[:])
            bias_t = cp.tile([128, N_FULL], F16)
            nc.gpsimd.partition_broadcast(bias_t[:], bias_row[:], channels=128)

            # --- x: transpose + quantize into SBUF-resident halves ---
            xT_res = []
            for g in range(M_SPLIT):
                xr = xrp.tile([128, KSUB, M_HALF], FP8, name=f"xr{g}")
                xT_res.append(xr)
                for ms in range(M_HALF // 128):
                    row0 = g * M_HALF + ms * 128
                    s = sp.tile([128, K], F32, tag="stripe")
                    nc.sync.dma_start(s[:], xs[row0:row0 + 128, :])
                    for kc in range(KSUB):
                        t = tpp.tile([128, 128], F32, tag="t")
                        nc.tensor.transpose(
                            t[:], s[:, kc * 128:(kc + 1) * 128], ident[:])
                        nc.vector.tensor_scalar_mul(
                            xr[:, kc, ms * 128:(ms + 1) * 128],
                            t[:], sc[:, 0:1])

            # --- DoubleRow matmul: stream gathered w^T once, both m-halves ---
            for h in range(2):
                for nb in range(N_CORES):
                    wt = wtp.tile([128, KSUB, N_HALF], FP8, tag="wt")
                    blk = wT_all[h].ap()[nb].rearrange(
                        "(ko p) n -> p ko n", p=128)
                    nc.sync.dma_start(wt[:], blk[:])
                    ncol0 = nb * N_LOC + h * N_HALF
                    for g in range(M_SPLIT):
                        xr = xT_res[g]
                        for mt in range(M_HALF // 128):
                            ps = mp.tile([128, N_TILE], F32, tag="ps")
                            for kp in range(KSUB // 2):
                                nc.tensor.matmul(
                                    ps[:],
                                    xr[:, 2 * kp:2 * kp + 2,
                                       mt * 128:(mt + 1) * 128],
                                    wt[:, 2 * kp:2 * kp + 2, :],
                                    start=(kp == 0), stop=(kp == KSUB // 2 - 1),
                                    perf_mode=mybir.MatmulPerfMode.DoubleRow)
                            ep = epp.tile([128, N_TILE], F16, tag="ep")
                            nc.vector.scalar_tensor_tensor(
                                out=ep[:], in0=ps[:], scalar=sc[:, 2:3],
                                in1=bias_t[:, ncol0:ncol0 + N_TILE],
                                op0=mybir.AluOpType.mult,
                                op1=mybir.AluOpType.add)
                            m0 = g * M_HALF + mt * 128
                            nc.sync.dma_start(
                                out[m0:m0 + 128, ncol0:ncol0 + N_TILE], ep[:])
    nc.compile()
    return nc


_CACHE = {}


def _get(name, builder):
    if name not in _CACHE:
        _CACHE[name] = builder()
    return _CACHE[name]


def kernel(x: np.ndarray, w: np.ndarray, bias: np.ndarray) -> np.ndarray:
    global LAST_EXEC_NS
    LAST_EXEC_NS = []
    x = np.asarray(x)
    w = np.asarray(w)
    bias = np.asarray(bias)
    assert x.shape[-1] == K and w.shape == (N_FULL, K) and bias.shape == (N_FULL,)
    x2d = np.ascontiguousarray(x.reshape(-1, K).astype(np.float32, copy=False))
    assert x2d.shape[0] == M_FULL
    w = np.ascontiguousarray(w.astype(np.float32, copy=False))
    bias = bias.astype(np.float16, copy=False)

    cores = list(range(N_CORES))

    # ---- launch A: partial absmax ----
    nc_a = _get("amax", _build_amax)
    ins_a = [
        {"xs": x2d[c * M_LOC:(c + 1) * M_LOC],
         "wl": w[c * N_LOC:(c + 1) * N_LOC]}
        for c in cores
    ]
    res_a = run_bass_kernel_spmd(nc_a, ins_a, core_ids=cores, trace=TRACE)
    if TRACE:
        LAST_EXEC_NS.append(res_a.exec_time_ns)
    parts = np.stack([res_a.results[c]["amax_out"][0] for c in cores])
    amax_x = np.float32(parts[:, 0].max())
    amax_w = np.float32(parts[:, 1].max())

    # ---- host: bit-exact scales (mirrors the jnp reference math) ----
    sx = np.float32(448.0) / np.maximum(amax_x, np.float32(1e-12))
    sw = np.float32(448.0) / np.maximum(amax_w, np.float32(1e-12))
    hx = sx * np.float32(0.5)          # exact halving: TRN e4m3 max is 240
    hw = sw * np.float32(0.5)
    inv_prod = np.float32(np.float32(1.0) / sx) * np.float32(np.float32(1.0) / sw)
    s_out = np.float32(inv_prod) * np.float32(4.0)
    scales = np.array([[hx, hw, s_out, 0.0]], dtype=np.float32)

    # ---- launch B ----
    nc_b = _get("main", _build_main)
    bias_row = np.ascontiguousarray(bias.reshape(1, N_FULL))
    ins_b = [
        {"xs": ins_a[c]["xs"], "wl": ins_a[c]["wl"],
         "bias_in": bias_row, "scales": scales}
        for c in cores
    ]
    res_b = run_bass_kernel_spmd(nc_b, ins_b, core_ids=cores, trace=TRACE)
    if TRACE:
        LAST_EXEC_NS.append(res_b.exec_time_ns)

    out = np.concatenate([res_b.results[c]["out"] for c in cores], axis=0)
    return out.reshape(*x.shape[:-1], N_FULL)

